# revision 42
# baseline (speedup 1.0000x reference)
"""Trainium2 Bass kernel for nn_MixedSparseGatedMLP (LoRA-augmented gated MLP).

Math (reference):
    y1 = x @ Wg + (x @ Ag) @ Bg
    y2 = x @ Wu + (x @ Au) @ Bu
    x3 = relu(y1) * y2
    y3 = x3 @ Wd + (x3 @ Ad) @ Bd

Strategy:
  - Fold the rank-16 LoRA factors into the dense weights on the host
    (exact fp32 algebra): Wg_eff = Wg + Ag@Bg, etc.  The device kernel is
    then a plain gated MLP with three dense matmuls.
  - Padding-free hybrid sharding over the intermediate dim I = 86*128:
    each core OWNS 10 i-chunks (processed for all 4096 tokens) and the 6
    leftover chunks are processed data-parallel: every core handles them
    for its OWN 512-token block only ("shared phase", fed by a per-core
    x_shared input).  Per-core work = 8 blocks*10 + 6 = 86 chunk-blocks,
    exactly 1/8 of the unpadded problem (the old I-padding to 88 chunks
    wasted 2.3% of PE time).
  - bf16 operands, fp32 PSUM accumulation, bf16 partial outputs (the
    8-way host reduction in fp32 keeps the rounding error ~4e-3).
  - All DRAM layouts are pre-tiled on the host so every DMA is a linear
    (or near-linear) copy into the exact SBUF layout the matmuls need.
  - Ramp: DMA flow starts ~9us into the kernel (fixed DGE latency) at
    ~330-430 GB/s aggregate.  Block 0 chunk 0's weights are split into
    256KB pieces and x into 512KB k-groups, need-ordered across the
    HWDGE (sync) and SWDGE (gpsimd) paths, so the first real matmul
    gates on ~0.75MB instead of ~6MB.  N=128 warmup matmuls keep the
    PE busy (and the HAM clock-gate warm) until the data lands.
"""

import os
import sys

for _p in ("/opt/trn_rl_repo", "/root/.axon_site/_ro/trn_rl_repo"):
    if os.path.isdir(_p) and _p not in sys.path:
        sys.path.append(_p)

import numpy as np
import ml_dtypes

# Problem shapes (hardcoded per contract)
B, S, H, I, R = 2, 2048, 4096, 11008, 16
NTOK = B * S              # 4096 tokens
NCORES = 8
CI = I // 128             # 86 i-chunks total (no padding: 86*128 == I)
C = 10                    # i-chunks OWNED per core (all tokens)
CS = CI - NCORES * C      # 6 leftover i-chunks, data-parallel over blocks
IS = C * 128              # 1280 owned intermediate columns per core
K = H // 128              # 32 h-chunks
TB = 512                  # token block
NB = NTOK // TB           # 8 token blocks
MT = TB // 128            # 4 token m-tiles per block
NH = H // 512             # 8 output n-tiles
KG = 4                    # x k-groups per block (1MB DMAs; HWDGE only
KS = K // KG              # sustains ~100GB/s below 1MB, ~180 at 1MB)

BF16 = ml_dtypes.bfloat16

# set by test.py for profiling; harness path leaves these as-is
TRACE = False
LAST_EXEC_TIME_NS = None
LAST_RESULTS = None


def _build_nc():
    import concourse.bacc as bacc
    import concourse.mybir as mybir
    import concourse.tile as tile

    bf16 = mybir.dt.bfloat16
    f32 = mybir.dt.float32

    nc = bacc.Bacc("TRN2", target_bir_lowering=False, debug=False)

    # DRAM parameters (host pre-tiled layouts; see kernel() for the math)
    x = nc.declare_dram_parameter("x", [NB, 128, K * TB], bf16, isOutput=False)
    xs = nc.declare_dram_parameter("xs", [128, K * TB], bf16, isOutput=False)
    wg = nc.declare_dram_parameter("wg", [C, 128, K * 128], bf16, isOutput=False)
    wu = nc.declare_dram_parameter("wu", [C, 128, K * 128], bf16, isOutput=False)
    wgs = nc.declare_dram_parameter("wgs", [CS, 128, K * 128], bf16, isOutput=False)
    wus = nc.declare_dram_parameter("wus", [CS, 128, K * 128], bf16, isOutput=False)
    wd = nc.declare_dram_parameter("wd", [C, 128, H], bf16, isOutput=False)
    wds = nc.declare_dram_parameter("wds", [CS, 128, H], bf16, isOutput=False)
    out = nc.declare_dram_parameter("out", [NTOK, H], bf16, isOutput=True)
    outs = nc.declare_dram_parameter("outs", [TB, H], bf16, isOutput=True)

    with tile.TileContext(nc) as tc:
        with tc.tile_pool(name="xp", bufs=1) as xp, \
             tc.tile_pool(name="wp0", bufs=1) as wp0, \
             tc.tile_pool(name="wp", bufs=4) as wp, \
             tc.tile_pool(name="wdp", bufs=1) as wdp, \
             tc.tile_pool(name="wdsp", bufs=3) as wdsp, \
             tc.tile_pool(name="x3p", bufs=2) as x3p, \
             tc.tile_pool(name="rp", bufs=2) as rp, \
             tc.tile_pool(name="op", bufs=4) as op, \
             tc.tile_pool(name="pgp", bufs=3, space="PSUM") as pgp, \
             tc.tile_pool(name="pup", bufs=2, space="PSUM") as pup, \
             tc.tile_pool(name="pdp", bufs=3, space="PSUM") as pdp:

            wdt = []
            # wd chunks issued per m-iteration of block 0: none during
            # the startup ramp (weight stream + x need full HBM BW), then
            # spread evenly so wu + wd on the SWDGE queue stay ~160GB/s
            # (2-per-m bursts pushed it to ~190 and slipped the weight
            # stream ~3us)
            WD_SCHED = [0, 0, 1, 1, 1, 1, 2, 2, 1, 1]

            # PE warmup: dependency-free matmuls run during the fixed
            # ~9us DMA-start latency + first-operand transfers; they lift
            # the HAM clock gate to 8/8 and keep the PE busy until the
            # first real operands land (~15us).  Measured pace ~440ns per
            # N=512 warm matmul (drain-serialized WAW chain).
            warm_in = rp.tile([128, TB], bf16, tag="warm")
            nc.any.memset(warm_in, 0.0)
            warm_ps = pdp.tile([128, TB], f32, tag="pd")

            def keepalive(n):
                # short dummy matmuls bridging predicted DMA-paced waits:
                # the HAM activity monitor re-throttles the PE clock to
                # 1.2GHz if a ~3.4us window sees idle.
                for _ in range(n):
                    nc.tensor.matmul(warm_ps[:, 0:128], warm_in[:, 0:128],
                                     warm_in[:, 0:128], start=True, stop=True)

            # 62 warmups x ~380-430ns end ~26-27us: covers the slowest
            # observed arrival of block 0 chunk 0's operands (per-queue
            # DMA rates swing ~2x run-to-run), so real matmuls always
            # start warm -- an idle-then-cold start costs ~7.5us, a
            # too-long warmup at most ~4us
            for _ in range(62):
                nc.tensor.matmul(warm_ps, warm_in[:, 0:128], warm_in,
                                 start=True, stop=True)

            def w_tile(src, m, eng=None):
                # monolithic 1MB weight DMAs: large transfers fan out over
                # more DMA engines and sustain ~2x the per-queue bandwidth
                # of 256KB tiles
                t = wp.tile([128, K * 128], bf16, tag="w")
                (eng or nc.sync).dma_start(t, src[m])
                return t

            def gate_up_chunk(wgt_pieces, wut_pieces, xbg, x3, m,
                              interleave=False, ka=None):
                # wgt/wut given as lists of (tile, k0, kn) pieces covering k
                def slices(pieces):
                    s = {}
                    for t, k0, kn in pieces:
                        for kk in range(kn):
                            s[k0 + kk] = t[:, kk * 128:(kk + 1) * 128]
                    return s

                gsl, usl = slices(wgt_pieces), slices(wut_pieces)
                g = pgp.tile([128, TB], f32, tag="pg")
                u = pup.tile([128, TB], f32, tag="pu")

                def mm(ps, sl, k):
                    gi, kg = divmod(k, KS)
                    nc.tensor.matmul(ps, sl[k],
                                     xbg[gi][:, kg * TB:(kg + 1) * TB],
                                     start=(k == 0), stop=(k == K - 1))

                if interleave:
                    # g/u alternated per x k-group: doubles the PE work
                    # unlocked per arrived byte during the block-0 DMA
                    # ramp, keeping the crawl gapless (no HAM re-throttle)
                    for gi in range(KG):
                        for kk in range(KS):
                            mm(g, gsl, gi * KS + kk)
                        for kk in range(KS):
                            mm(u, usl, gi * KS + kk)
                        if ka and gi in ka:
                            keepalive(ka[gi])
                else:
                    for ps, sl in ((g, gsl), (u, usl)):
                        for k in range(K):
                            mm(ps, sl, k)
                            if ka and ps is g and k in ka:
                                keepalive(ka[k])
                # x3 = relu(g) * u ; DVE may read only one PSUM input,
                # so relu lands in SBUF via ACT first.
                r = rp.tile([128, TB], bf16, tag="r")
                nc.scalar.activation(r, g, mybir.ActivationFunctionType.Relu)
                nc.vector.tensor_mul(x3[:, m * TB:(m + 1) * TB], r, u)

            for b in range(NB):
                # x block in KG groups: [128 h-in-chunk, (k, t)] bf16
                xbg = [None] * KG
                w0 = None
                if b == 0:
                    # Startup ramp: per-queue DMA is only ~165GB/s, so
                    # block 0's critical ~6MB is spread across FOUR DGE
                    # queues (sync/gpsimd + the otherwise-idle scalar and
                    # vector queues) in need-time order.  First matmul
                    # gates on wg0-half + xb0 on separate queues (~1MB).
                    def xg(gi, eng):
                        t = xp.tile([128, KS * TB], bf16, tag=f"xb{gi}")
                        eng.dma_start(t, x[b][:, gi * KS * TB:(gi + 1) * KS * TB])
                        xbg[gi] = t

                    def wpiece(src, j, eng):
                        t = wp0.tile([128, 16 * 128], bf16, tag=f"{src}p{j}")
                        s = wg if src == "g" else wu
                        eng.dma_start(t, s[0][:, j * 16 * 128:(j + 1) * 16 * 128])
                        return (t, j * 16, 16)

                    # Monolithic 1MB DMAs split across both DGE paths in
                    # need order; ~6MB lands by ~26us, right as the
                    # warmup chain drains (early-start variants lost more
                    # to HAM re-throttle + small-DMA bandwidth than they
                    # saved).
                    # sync+scalar HWDGE share only ~100-200GB/s while
                    # gpsimd SWDGE bursts ~250-300GB/s: wg0/wu0 go as
                    # queue-parallel 512KB halves on the two HWDGE queues
                    # (arrive ~14-18us), ALL x k-groups stream on gpsimd
                    # in consumption order.
                    t = wp0.tile([128, K * 128], bf16, tag="g0")
                    nc.sync.dma_start(t, wg[0])
                    wgt0 = [(t, 0, K)]
                    xg(0, nc.gpsimd)
                    xg(1, nc.gpsimd)
                    t = wp0.tile([128, K * 128], bf16, tag="u0")
                    nc.scalar.dma_start(t, wu[0])
                    wut0 = [(t, 0, K)]
                    xg(2, nc.gpsimd)
                    xg(3, nc.gpsimd)
                    w0 = (wgt0, wut0)
                else:
                    for gi in range(KG):
                        t = xp.tile([128, KS * TB], bf16, tag=f"xb{gi}")
                        nc.sync.dma_start(t, x[b][:, gi * KS * TB:(gi + 1) * KS * TB])
                        xbg[gi] = t

                # x3^T for this block: [128 i-in-chunk, (c, t)] bf16
                x3 = x3p.tile([128, C * TB], bf16, tag="x3")

                # ---- gate / up projections + gating, per i-chunk m ----
                for m in range(C):
                    if m == 0 and w0 is not None:
                        # plain g-then-u order (matches arrival order);
                        # small keepalives bridge xg2/xg3 arrival jitter
                        gate_up_chunk(w0[0], w0[1], xbg, x3, m,
                                      ka={15: 5, 23: 5})
                        continue
                    else:
                        # block 0 has no previous down-phase to build DMA
                        # lead under; spread its weight stream across all
                        # three DGE paths (sync HWDGE alone degrades to
                        # ~60-105GB/s and stalled the m=3-4 gate weights)
                        if b == 0:
                            ge = nc.scalar if m % 2 else nc.sync
                            wgt_p = [(w_tile(wg, m, ge), 0, K)]
                            wut_p = [(w_tile(wu, m, nc.gpsimd), 0, K)]
                        else:
                            wgt_p = [(w_tile(wg, m), 0, K)]
                            wut_p = [(w_tile(wu, m), 0, K)]

                    gate_up_chunk(wgt_p, wut_p, xbg, x3, m)

                    if b == 0:
                        # Wd (own chunks) stays SBUF-resident for the whole
                        # kernel (10 x 1MB).  Preload on the software-DGE
                        # path, back-loaded per WD_SCHED: all chunks must
                        # land by block 0's down phase, but issuing them
                        # early floods HBM during the startup ramp.
                        # (Moving any of these to the scalar HWDGE queue
                        # starves the wg/wu weight streams -- measured
                        # +26us.)
                        for _ in range(WD_SCHED[m]):
                            c = len(wdt)
                            t = wdp.tile([128, H], bf16, tag=f"wd{c}")
                            nc.gpsimd.dma_start(t, wd[c])
                            wdt.append(t)

                # ---- down projection: out[tok, h] partial ----
                for mt in range(MT):
                    for n in range(NH):
                        d = pdp.tile([128, 512], f32, tag="pd")
                        for c in range(C):
                            nc.tensor.matmul(
                                d,
                                x3[:, c * TB + mt * 128: c * TB + (mt + 1) * 128],
                                wdt[c][:, n * 512:(n + 1) * 512],
                                start=(c == 0), stop=(c == C - 1),
                            )
                        o = op.tile([128, 512], bf16, tag="o")
                        nc.scalar.copy(o, d)
                        row = b * TB + mt * 128
                        # store via SWDGE: keeps HWDGE free for the
                        # x/weight prefetches that gate the next block.
                        nc.gpsimd.dma_start(
                            out[row:row + 128, n * 512:(n + 1) * 512], o
                        )

            # ---- shared phase: the 6 leftover i-chunks, this core's own
            # 512-token block only (fed by the per-core xs input). ----
            # xs split across both main queues so the shared-phase weight
            # stream (wgs on sync) isn't delayed behind 4MB of x
            xsg = []
            for gi in range(KG):
                t = xp.tile([128, KS * TB], bf16, tag=f"xb{gi}")
                eng = nc.sync if gi % 2 == 0 else nc.gpsimd
                eng.dma_start(t, xs[:, gi * KS * TB:(gi + 1) * KS * TB])
                xsg.append(t)

            x3s = x3p.tile([128, CS * TB], bf16, tag="x3")
            for m in range(CS):
                # split gate/up weight streams across both DGE paths:
                # 2MB per 13.8us chunk exceeds a single queue's ~165GB/s
                wgt_p = [(w_tile(wgs, m), 0, K)]
                wut_p = [(w_tile(wus, m, nc.gpsimd), 0, K)]
                gate_up_chunk(wgt_p, wut_p, xsg, x3s, m)

            # shared down: n-outer so the wds weights stream through a
            # small pool of [128,512] slices instead of living resident;
            # slices alternate between the gpsimd and scalar queues (one
            # queue can't sustain 148GB/s of 128KB transfers).
            for n in range(NH):
                wdst = []
                for c in range(CS):
                    t = wdsp.tile([128, 512], bf16, tag=f"wds{c}")
                    eng = nc.gpsimd if c % 2 == 0 else nc.scalar
                    eng.dma_start(t, wds[c][:, n * 512:(n + 1) * 512])
                    wdst.append(t)
                for mt in range(MT):
                    d = pdp.tile([128, 512], f32, tag="pd")
                    for c in range(CS):
                        nc.tensor.matmul(
                            d,
                            x3s[:, c * TB + mt * 128: c * TB + (mt + 1) * 128],
                            wdst[c],
                            start=(c == 0), stop=(c == CS - 1),
                        )
                    o = op.tile([128, 512], bf16, tag="o")
                    nc.scalar.copy(o, d)
                    # HWDGE is idle in the shared down phase, and its
                    # lower latency trims the kernel tail.
                    nc.sync.dma_start(
                        outs[mt * 128:(mt + 1) * 128, n * 512:(n + 1) * 512], o
                    )

    nc.compile()
    return nc


def _prep_inputs(x1, w_gate, w_gate_lora_a, w_gate_lora_b,
                 w_up, w_up_lora_a, w_up_lora_b,
                 w_down, w_down_lora_a, w_down_lora_b):
    """Fold LoRA, shard per core (10 own + 6 shared chunks), pre-tile."""
    f32 = np.float32
    x1 = np.asarray(x1, f32)
    wg_eff = np.asarray(w_gate, f32) + np.asarray(w_gate_lora_a, f32) @ np.asarray(w_gate_lora_b, f32)
    wu_eff = np.asarray(w_up, f32) + np.asarray(w_up_lora_a, f32) @ np.asarray(w_up_lora_b, f32)
    wd_eff = np.asarray(w_down, f32) + np.asarray(w_down_lora_a, f32) @ np.asarray(w_down_lora_b, f32)

    # x tile layout: x_tiled[b, p, k, t] = x2d[b*TB + t, k*128 + p]
    x2d = x1.reshape(NTOK, H)
    x_tiled = np.ascontiguousarray(
        x2d.reshape(NB, TB, K, 128).transpose(0, 3, 2, 1)
    ).astype(BF16).reshape(NB, 128, K * TB)

    def wgu_tile(w, sl, c):
        # [m, p, k, i] = w[k*128+p, sl.start + m*128 + i]
        return np.ascontiguousarray(
            w[:, sl].reshape(K, 128, c, 128).transpose(2, 1, 0, 3)
        ).astype(BF16).reshape(c, 128, K * 128)

    sh = slice(NCORES * IS, I)       # the 6 shared chunks
    wgs_t = wgu_tile(wg_eff, sh, CS)
    wus_t = wgu_tile(wu_eff, sh, CS)
    wds_t = wd_eff[sh, :].reshape(CS, 128, H).astype(BF16)

    in_maps = []
    for ci in range(NCORES):
        sl = slice(ci * IS, (ci + 1) * IS)
        in_maps.append({
            "x": x_tiled,
            "xs": x_tiled[ci],
            "wg": wgu_tile(wg_eff, sl, C),
            "wu": wgu_tile(wu_eff, sl, C),
            "wgs": wgs_t,
            "wus": wus_t,
            "wd": wd_eff[sl, :].reshape(C, 128, H).astype(BF16),
            "wds": wds_t,
        })
    return in_maps


def _emulate(in_maps):
    """Numpy emulation of the device math (bf16 operands, fp32 accum).
    Validates the host-side tilings and predicts the on-device accuracy."""
    f32 = np.float32
    acc = np.zeros((NTOK, H), f32)
    # reconstruct x2d (bf16-rounded) from the tiled layout
    xt = in_maps[0]["x"].reshape(NB, 128, K, TB)
    x2d = xt.transpose(0, 3, 2, 1).reshape(NTOK, H).astype(f32)

    def untile_wgu(wt, c):
        return wt.reshape(c, 128, K, 128).transpose(2, 1, 0, 3).reshape(H, c * 128).astype(f32)

    def mlp(x, wg2, wu2, wd2):
        y1 = x @ wg2
        y2 = x @ wu2
        r = np.maximum(y1, 0).astype(BF16).astype(f32)
        x3 = (r * y2).astype(BF16).astype(f32)
        return (x3 @ wd2).astype(BF16).astype(f32)

    for ci, m in enumerate(in_maps):
        acc += mlp(x2d, untile_wgu(m["wg"], C), untile_wgu(m["wu"], C),
                   m["wd"].reshape(IS, H).astype(f32))
        rows = slice(ci * TB, (ci + 1) * TB)
        acc[rows] += mlp(x2d[rows], untile_wgu(m["wgs"], CS),
                         untile_wgu(m["wus"], CS),
                         m["wds"].reshape(CS * 128, H).astype(f32))
    return acc.reshape(B, S, H)


def kernel(**inputs):
    global LAST_EXEC_TIME_NS, LAST_RESULTS
    in_maps = _prep_inputs(**inputs)

    if os.environ.get("KERNEL_EMULATE"):
        return _emulate(in_maps)

    from concourse.bass_utils import run_bass_kernel_spmd

    nc = _build_nc()
    res = run_bass_kernel_spmd(nc, in_maps, list(range(NCORES)), trace=TRACE)
    LAST_EXEC_TIME_NS = res.exec_time_ns
    LAST_RESULTS = res

    acc = np.zeros((NTOK, H), np.float32)
    for ci, r in enumerate(res.results):
        acc += r["out"].astype(np.float32)
        rows = slice(ci * TB, (ci + 1) * TB)
        acc[rows] += r["outs"].astype(np.float32)
    return acc.reshape(B, S, H)
